# revision 2
# baseline (speedup 1.0000x reference)
"""Bilateral anti-alias filter on Trainium2, 8-core data parallel.

Full inputs: images [16,3,512,512] f32, spatial_kernel [5,5] f32.
Shards the batch over 8 NeuronCores (2 images each), runs a Bass/Tile
kernel per core, gathers the full output.

Math (per pixel, K=5, sigma_i=0.1):
  w_t = s_t * exp(-50*(p_t - c)^2),  out = sum(w_t*p_t)/(sum(w_t)+eps)
Pair symmetry: w for tap -(di,dj) at pixel r equals w for tap +(di,dj)
at pixel r-(di,dj).  So only 12 weight planes are computed; each
contributes 4 terms (direct + shifted, for numerator + denominator),
accumulated on the TensorEngine via 0/1 shift-matrix matmuls into PSUM.
"""
import sys

sys.path.insert(0, "/opt/trn_rl_repo")

import numpy as np
import ml_dtypes
from contextlib import ExitStack

import concourse.bass as bass
import concourse.tile as tile
from concourse import bacc, mybir
from concourse.bass_utils import run_bass_kernel_spmd

f32 = mybir.dt.float32
bf16 = mybir.dt.bfloat16
AF = mybir.ActivationFunctionType
Alu = mybir.AluOpType

N_CORES = 8
B_FULL, C, H, W = 16, 3, 512, 512
B_SH = B_FULL // N_CORES  # 2 images per core
KK = 5
PAD = KK // 2  # 2
INV2SIG2 = 1.0 / (2.0 * 0.1 * 0.1)  # 50.0

# 12 pairs (di, dj) with di >= 0, lexicographically positive
PAIRS = [
    (0, 1), (0, 2),
    (1, -2), (1, -1), (1, 0), (1, 1), (1, 2),
    (2, -2), (2, -1), (2, 0), (2, 1), (2, 2),
]

GROUPS = [
    [(0, 1), (1, 0)], [(1, -1), (1, 1)], [(0, 2), (2, 0)],
    [(1, -2), (1, 2)], [(2, -1), (2, 1)], [(2, -2), (2, 2)],
]
SUB_BF16 = True   # compute d = p - c in bf16 (DVE 2x mode)
SQ_GPS = False     # squares on GpSimd instead of ACT
NOUT = 124  # output rows per tile
NG = 128    # grid partitions (= NOUT + 4)
WB = W + 4  # 516: padded col buffer, idx j <-> col j-2


def _row_bands(h):
    """Tile start rows: uniform NOUT-row bands; last band overlaps upward."""
    bands = list(range(0, h - NOUT + 1, NOUT))
    if bands[-1] != h - NOUT:
        bands.append(h - NOUT)
    return bands


def _reflect_runs(v0, v1, h):
    """Split virtual row range [v0, v1] into runs of physical rows.
    Returns list of (p_offset, phys_start, count, step) with step +-1."""
    runs = []
    v = v0
    while v <= v1:
        if v < 0:
            # rows v..min(-1, v1) -> physical -v .. (reversed, step -1)
            e = min(-1, v1)
            runs.append((v - v0, -v, e - v + 1, -1))
            v = e + 1
        elif v >= h:
            e = v1
            runs.append((v - v0, 2 * h - 2 - v, e - v + 1, -1))
            v = e + 1
        else:
            e = min(h - 1, v1)
            runs.append((v - v0, v, e - v + 1, 1))
            v = e + 1
    return runs


def build_bilateral(nc, biases, h=H, w=W, b_sh=B_SH, c=C):
    """Emit the per-core program into nc (a Bacc). biases[(di,dj)] = ln s."""
    wb = w + 4
    img_d = nc.dram_tensor("images", [b_sh, c, h, w], f32, kind="ExternalInput").ap()
    shifts_d = nc.dram_tensor("shifts", [NG, 5, NOUT], bf16, kind="ExternalInput").ap()
    out_d = nc.dram_tensor("out", [b_sh, c, h, w], f32, kind="ExternalOutput").ap()

    # const APs for activation biases
    for val in sorted(set(biases.values()) | {1.0}):
        key = (f32, val)
        if key not in nc.const_aps.aps:
            t = nc.alloc_sbuf_tensor(f"cbias-{val}", [128, 1], f32)
            nc.gpsimd.memset(t.ap(), val)
            nc.const_aps.aps[key] = t.ap()
    nc.all_engine_barrier()

    # Pre-place one ACT table load for a set containing Square+Exp+Ln.
    # Without this the compiler's greedy chooser thrashes between
    # exp_and_others and natural_log (2 reloads/tile, ~2.7us each).
    from concourse.hw_specs import get_activation_tables

    set_names = list(get_activation_tables(nc.m.arch))
    nc.scalar.add_instruction(
        mybir.InstLoadActFuncSet(
            name=nc.get_next_instruction_name(),
            act_func_set_id=set_names.index("natural_log_exp_and_others"),
            ins=[],
            outs=[],
        )
    )

    bands = _row_bands(h)

    with tile.TileContext(nc) as tc, ExitStack() as ctx:
        consts = ctx.enter_context(tc.tile_pool(name="consts", bufs=1))
        imgs_f = ctx.enter_context(tc.tile_pool(name="imgs_f", bufs=2))
        imgs_b = ctx.enter_context(tc.tile_pool(name="imgs_b", bufs=2))
        planes = ctx.enter_context(tc.tile_pool(name="planes", bufs=2))
        prods = ctx.enter_context(tc.tile_pool(name="prods", bufs=4))
        finals = ctx.enter_context(tc.tile_pool(name="finals", bufs=1))
        psums = ctx.enter_context(tc.tile_pool(name="psums", bufs=1, space="PSUM"))

        shifts = consts.tile([NG, 5, NOUT], bf16)
        nc.sync.dma_start(shifts[:], shifts_d[:])

        for bi in range(b_sh):
            for r0 in bands:
                # ---- load 3 row-shifted reflect-padded image copies ----
                ifs = []
                for s in range(3):
                    t = imgs_f.tile([NG, c, wb], f32, tag=f"i{s}f")
                    refl_rows = []
                    for (po, ps, cnt, step) in _reflect_runs(
                        r0 - 2 + s, r0 - 2 + s + NG - 1, h
                    ):
                        if step == 1:
                            src = img_d[bi, :, ps : ps + cnt, :]
                            nc.sync.dma_start(
                                t[po : po + cnt, :, 2 : 2 + w],
                                src.rearrange("c r n -> r c n"),
                            )
                        else:
                            for k in range(cnt):
                                refl_rows.append((po + k, ps - k))
                    # reflect rows duplicate rows already in the tile:
                    # single-partition SBUF self-copies (main cols)
                    for (p_dst, phys) in refl_rows:
                        p_src = phys - (r0 - 2 + s)
                        nc.sync.dma_start(
                            t[p_dst : p_dst + 1, :, 2 : 2 + w],
                            t[p_src : p_src + 1, :, 2 : 2 + w],
                        )
                    # reflect pad cols: in-tile duplicates, tiny GPSIMD copies
                    for (j, jsrc) in ((0, 4), (1, 3), (2 + w, w), (3 + w, w - 1)):
                        nc.gpsimd.tensor_copy(
                            t[:, :, j : j + 1], t[:, :, jsrc : jsrc + 1]
                        )
                    ifs.append(t)

                # bf16 copies: A (cast), B (A shifted 1 col, via DMA)
                ibA, ibB = [], []
                for s in range(3):
                    a = imgs_b.tile([NG, c, wb], bf16, tag=f"i{s}bA")
                    nc.vector.tensor_copy(a[:], ifs[s][:])
                    ibA.append(a)
                    b = imgs_b.tile([NG, c, wb], bf16, tag=f"i{s}bB")
                    nc.sync.dma_start(b[:, :, 0 : wb - 1], a[:, :, 1:wb])
                    ibB.append(b)

                # ---- PSUM accumulators (512-wide per channel = one 2KB
                # zero-region/bank per channel, required for start flags) ----
                pw = psums.tile([NOUT, c, 512], f32, tag="pw")
                pa = psums.tile([NOUT, c, 512], f32, tag="pa")

                n_mm = len(PAIRS) * 2  # mms per psum target per channel
                mm_i = 0
                for grp in GROUPS:
                    G = len(grp)
                    dg = planes.tile([NG, G * c, w + 2], bf16, tag="d")
                    for gi, (di, dj) in enumerate(grp):
                        cP = -2 if dj > 0 else 0
                        if dj % 2 == 0:
                            dsrc = ibA[di][:, :, cP + dj + 2 : cP + dj + 4 + w]
                        else:
                            dsrc = ibB[di][:, :, cP + dj + 1 : cP + dj + 3 + w]
                        nc.vector.tensor_tensor(
                            dg[:, gi * c : (gi + 1) * c, :], dsrc,
                            ibA[0][:, :, cP + 2 : cP + 4 + w], Alu.subtract,
                        )
                    sqg = planes.tile([NG, G * c, w + 2], f32, tag="sq")
                    nc.scalar.activation(sqg[:], dg[:], AF.Square)
                    wg = planes.tile([NG, G * c, w + 2], bf16, tag="w")
                    nc.scalar.activation(
                        wg[:], sqg[:], AF.Exp,
                        bias=biases[grp[0]], scale=-INV2SIG2,
                    )
                    for gi, (di, dj) in enumerate(grp):
                        cP = -2 if dj > 0 else 0
                        wp = wg[:, gi * c : (gi + 1) * c, :]
                        # Z = w * img(center rows),  Y = w * img(shifted)
                        zp = prods.tile([NG, c, w + 2], bf16, tag="z")
                        nc.vector.tensor_tensor(
                            zp[:], wp[:], ibA[0][:, :, cP + 2 : cP + 2 + w + 2], Alu.mult
                        )
                        yp = prods.tile([NG, c, w], bf16, tag="y")
                        if dj % 2 == 0:
                            ysrc = ibA[di][:, :, dj + 2 : dj + 2 + w]
                        else:
                            ysrc = ibB[di][:, :, dj + 1 : dj + 1 + w]
                        nc.vector.tensor_tensor(
                            yp[:], wp[:, :, -cP : -cP + w], ysrc, Alu.mult
                        )

                        # ---- 4 matmuls per channel ----
                        s_dir = 2
                        s_sh = 2 - di
                        first = mm_i == 0
                        last = mm_i == n_mm - 2  # this pair adds 2 mms per target
                        w_merged = dj == 0  # dir+shifted share the rhs window
    # (merged lhsT idx: di=1 -> 3, di=2 -> 4)
                        for ch in range(c):
                            nc.tensor.matmul(
                                pw[:, ch, 0:w],
                                shifts[:, (2 + di) if w_merged else s_dir, :],
                                wp[:, ch, -cP : -cP + w],
                                start=first,
                                stop=last if w_merged else False,
                            )
                            nc.tensor.matmul(
                                pa[:, ch, 0:w],
                                shifts[:, s_dir, :],
                                yp[:, ch, :],
                                start=first,
                                stop=False,
                            )
                        for ch in range(c):
                            if not w_merged:
                                nc.tensor.matmul(
                                    pw[:, ch, 0:w],
                                    shifts[:, s_sh, :],
                                    wp[:, ch, -dj - cP : -dj - cP + w],
                                    start=False,
                                    stop=last,
                                )
                            nc.tensor.matmul(
                                pa[:, ch, 0:w],
                                shifts[:, s_sh, :],
                                zp[:, ch, -dj - cP : -dj - cP + w],
                                start=False,
                                stop=last,
                            )
                        mm_i += 2

                # ---- finalize: out = (pa + center) * exp(-ln(pw + 1)) ----
                lnv = finals.tile([NOUT, c, w], f32, tag="lnv")
                nc.scalar.activation(lnv[:], pw[:, :, 0:w], AF.Ln, bias=1.0)
                rec = finals.tile([NOUT, c, w], f32, tag="rec")
                nc.scalar.activation(rec[:], lnv[:], AF.Exp, scale=-1.0)
                acct = finals.tile([NOUT, c, w], f32, tag="acct")
                nc.vector.tensor_tensor(
                    acct[:], pa[:, :, 0:w], ifs[2][0:NOUT, :, 2 : 2 + w], Alu.add
                )
                res = finals.tile([NOUT, c, w], f32, tag="res")
                nc.vector.tensor_tensor(res[:], acct[:], rec[:], Alu.mult)
                # overlap band: only write rows not already written by the
                # previous band (avoids DRAM WAW serialization)
                oo = 0 if r0 == bands[0] else max(0, prev_end - r0)
                nc.sync.dma_start(
                    out_d[bi, :, r0 + oo : r0 + NOUT, :].rearrange("c r n -> r c n"),
                    res[oo:NOUT],
                )
                prev_end = r0 + NOUT
    return nc


def _shift_mats():
    s = np.zeros((NG, 5, NOUT), dtype=ml_dtypes.bfloat16)
    for k in range(3):
        for m in range(NOUT):
            s[m + k, k, m] = 1.0
    # merged direct+shifted for dj=0 planes: idx 3 = L2+L1, idx 4 = L2+L0
    s[:, 3] = s[:, 2] + s[:, 1]
    s[:, 4] = s[:, 2] + s[:, 0]
    return s


def make_program(spatial_kernel):
    biases = {}
    for (di, dj) in PAIRS:
        v = float(np.float32(np.log(np.float32(spatial_kernel[2 + di, 2 + dj]))))
        biases[(di, dj)] = v
    nc = bacc.Bacc("TRN2", target_bir_lowering=False, debug=False)
    build_bilateral(nc, biases)
    nc.compile()
    return nc


def kernel(images, spatial_kernel):
    images = np.asarray(images, dtype=np.float32)
    spatial_kernel = np.asarray(spatial_kernel, dtype=np.float32)
    nc = make_program(spatial_kernel)
    shifts = _shift_mats()
    in_maps = [
        {"images": images[i * B_SH : (i + 1) * B_SH], "shifts": shifts}
        for i in range(N_CORES)
    ]
    res = run_bass_kernel_spmd(nc, in_maps, core_ids=list(range(N_CORES)))
    return np.concatenate([res.results[i]["out"] for i in range(N_CORES)], axis=0)



# revision 3
# speedup vs baseline: 1.0911x; 1.0911x over previous
"""Bilateral anti-alias filter on Trainium2, 8-core data parallel.

Full inputs: images [16,3,512,512] f32, spatial_kernel [5,5] f32.
Shards the batch over 8 NeuronCores (2 images each), runs a Bass/Tile
kernel per core, gathers the full output.

Math (per pixel, K=5, sigma_i=0.1):
  w_t = s_t * exp(-50*(p_t - c)^2),  out = sum(w_t*p_t)/(sum(w_t)+eps)
Pair symmetry: w for tap -(di,dj) at pixel r equals w for tap +(di,dj)
at pixel r-(di,dj).  So only 12 weight planes are computed; each
contributes 4 terms (direct + shifted, for numerator + denominator),
accumulated on the TensorEngine via 0/1 shift-matrix matmuls into PSUM.
"""
import sys

sys.path.insert(0, "/opt/trn_rl_repo")

import numpy as np
import ml_dtypes
from contextlib import ExitStack

import concourse.bass as bass
import concourse.tile as tile
from concourse import bacc, mybir
from concourse.bass_utils import run_bass_kernel_spmd

f32 = mybir.dt.float32
bf16 = mybir.dt.bfloat16
AF = mybir.ActivationFunctionType
Alu = mybir.AluOpType

N_CORES = 8
B_FULL, C, H, W = 16, 3, 512, 512
B_SH = B_FULL // N_CORES  # 2 images per core
KK = 5
PAD = KK // 2  # 2
INV2SIG2 = 1.0 / (2.0 * 0.1 * 0.1)  # 50.0

# 12 pairs (di, dj) with di >= 0, lexicographically positive
PAIRS = [
    (0, 1), (0, 2),
    (1, -2), (1, -1), (1, 0), (1, 1), (1, 2),
    (2, -2), (2, -1), (2, 0), (2, 1), (2, 2),
]

GROUPS = [
    [(0, 1), (1, 0)], [(1, -1), (1, 1)], [(0, 2), (2, 0)],
    [(1, -2), (1, 2)], [(2, -1), (2, 1)], [(2, -2), (2, 2)],
]
SUB_BF16 = True   # compute d = p - c in bf16 (DVE 2x mode)
SQ_GPS = False     # squares on GpSimd instead of ACT
NOUT = 124  # output rows per tile
NG = 128    # grid partitions (= NOUT + 4)
WB = W + 4  # 516: padded col buffer, idx j <-> col j-2


def _row_bands(h):
    """Tile start rows: uniform NOUT-row bands; last band overlaps upward."""
    bands = list(range(0, h - NOUT + 1, NOUT))
    if bands[-1] != h - NOUT:
        bands.append(h - NOUT)
    return bands


def _reflect_runs(v0, v1, h):
    """Split virtual row range [v0, v1] into runs of physical rows.
    Returns list of (p_offset, phys_start, count, step) with step +-1."""
    runs = []
    v = v0
    while v <= v1:
        if v < 0:
            # rows v..min(-1, v1) -> physical -v .. (reversed, step -1)
            e = min(-1, v1)
            runs.append((v - v0, -v, e - v + 1, -1))
            v = e + 1
        elif v >= h:
            e = v1
            runs.append((v - v0, 2 * h - 2 - v, e - v + 1, -1))
            v = e + 1
        else:
            e = min(h - 1, v1)
            runs.append((v - v0, v, e - v + 1, 1))
            v = e + 1
    return runs


def build_bilateral(nc, biases, h=H, w=W, b_sh=B_SH, c=C):
    """Emit the per-core program into nc (a Bacc). biases[(di,dj)] = ln s."""
    wb = w + 4
    img_d = nc.dram_tensor("images", [b_sh, c, h, w], f32, kind="ExternalInput").ap()
    shifts_d = nc.dram_tensor("shifts", [NG, 5, NOUT], bf16, kind="ExternalInput").ap()
    out_d = nc.dram_tensor("out", [b_sh, c, h, w], f32, kind="ExternalOutput").ap()

    # const APs for activation biases
    for val in sorted(set(biases.values()) | {1.0}):
        key = (f32, val)
        if key not in nc.const_aps.aps:
            t = nc.alloc_sbuf_tensor(f"cbias-{val}", [128, 1], f32)
            nc.gpsimd.memset(t.ap(), val)
            nc.const_aps.aps[key] = t.ap()
    nc.all_engine_barrier()

    # Pre-place one ACT table load for a set containing Square+Exp+Ln.
    # Without this the compiler's greedy chooser thrashes between
    # exp_and_others and natural_log (2 reloads/tile, ~2.7us each).
    from concourse.hw_specs import get_activation_tables

    set_names = list(get_activation_tables(nc.m.arch))
    if False:
        nc.scalar.add_instruction(
            mybir.InstLoadActFuncSet(
                name=nc.get_next_instruction_name(),
                act_func_set_id=set_names.index("natural_log_exp_and_others"),
                ins=[],
                outs=[],
            )
        )

    bands = _row_bands(h)

    with tile.TileContext(nc) as tc, ExitStack() as ctx:
        consts = ctx.enter_context(tc.tile_pool(name="consts", bufs=1))
        imgs_f = ctx.enter_context(tc.tile_pool(name="imgs_f", bufs=2))
        imgs_b = ctx.enter_context(tc.tile_pool(name="imgs_b", bufs=2))
        planes = ctx.enter_context(tc.tile_pool(name="planes", bufs=2))
        prods = ctx.enter_context(tc.tile_pool(name="prods", bufs=4))
        finals = ctx.enter_context(tc.tile_pool(name="finals", bufs=1))
        psums = ctx.enter_context(tc.tile_pool(name="psums", bufs=1, space="PSUM"))

        shifts = consts.tile([NG, 5, NOUT], bf16)
        nc.sync.dma_start(shifts[:], shifts_d[:])

        for bi in range(b_sh):
            for r0 in bands:
                # ---- load 3 row-shifted reflect-padded image copies ----
                ifs = []
                for s in range(3):
                    t = imgs_f.tile([NG, c, wb], f32, tag=f"i{s}f")
                    refl_rows = []
                    for (po, ps, cnt, step) in _reflect_runs(
                        r0 - 2 + s, r0 - 2 + s + NG - 1, h
                    ):
                        if step == 1:
                            src = img_d[bi, :, ps : ps + cnt, :]
                            nc.sync.dma_start(
                                t[po : po + cnt, :, 2 : 2 + w],
                                src.rearrange("c r n -> r c n"),
                            )
                        else:
                            for k in range(cnt):
                                refl_rows.append((po + k, ps - k))
                    # reflect rows duplicate rows already in the tile:
                    # single-partition SBUF self-copies (main cols)
                    for (p_dst, phys) in refl_rows:
                        p_src = phys - (r0 - 2 + s)
                        nc.sync.dma_start(
                            t[p_dst : p_dst + 1, :, 2 : 2 + w],
                            t[p_src : p_src + 1, :, 2 : 2 + w],
                        )
                    # reflect pad cols: in-tile duplicates, tiny GPSIMD copies
                    for (j, jsrc) in ((0, 4), (1, 3), (2 + w, w), (3 + w, w - 1)):
                        nc.gpsimd.tensor_copy(
                            t[:, :, j : j + 1], t[:, :, jsrc : jsrc + 1]
                        )
                    ifs.append(t)

                # bf16 copies: A (cast), B (A shifted 1 col, via DMA)
                ibA, ibB = [], []
                for s in range(3):
                    a = imgs_b.tile([NG, c, wb], bf16, tag=f"i{s}bA")
                    nc.vector.tensor_copy(a[:], ifs[s][:])
                    ibA.append(a)
                    b = imgs_b.tile([NG, c, wb], bf16, tag=f"i{s}bB")
                    nc.sync.dma_start(b[:, :, 0 : wb - 1], a[:, :, 1:wb])
                    ibB.append(b)

                # ---- PSUM accumulators (512-wide per channel = one 2KB
                # zero-region/bank per channel, required for start flags) ----
                pw = psums.tile([NOUT, c, 512], f32, tag="pw")
                pa = psums.tile([NOUT, c, 512], f32, tag="pa")

                n_mm = len(PAIRS) * 2  # mms per psum target per channel
                mm_i = 0
                for grp in GROUPS:
                    G = len(grp)
                    dg = planes.tile([NG, G * c, w + 2], bf16, tag="d")
                    for gi, (di, dj) in enumerate(grp):
                        cP = -2 if dj > 0 else 0
                        if dj % 2 == 0:
                            dsrc = ibA[di][:, :, cP + dj + 2 : cP + dj + 4 + w]
                        else:
                            dsrc = ibB[di][:, :, cP + dj + 1 : cP + dj + 3 + w]
                        nc.vector.tensor_tensor(
                            dg[:, gi * c : (gi + 1) * c, :], dsrc,
                            ibA[0][:, :, cP + 2 : cP + 4 + w], Alu.subtract,
                        )
                    sqg = planes.tile([NG, G * c, w + 2], f32, tag="sq")
                    nc.scalar.activation(sqg[:], dg[:], AF.Square)
                    wg = planes.tile([NG, G * c, w + 2], bf16, tag="w")
                    nc.scalar.activation(
                        wg[:], sqg[:], AF.Exp,
                        bias=biases[grp[0]], scale=-INV2SIG2,
                    )
                    for gi, (di, dj) in enumerate(grp):
                        cP = -2 if dj > 0 else 0
                        wp = wg[:, gi * c : (gi + 1) * c, :]
                        # Z = w * img(center rows),  Y = w * img(shifted)
                        zp = prods.tile([NG, c, w + 2], bf16, tag="z")
                        nc.vector.tensor_tensor(
                            zp[:], wp[:], ibA[0][:, :, cP + 2 : cP + 2 + w + 2], Alu.mult
                        )
                        yp = prods.tile([NG, c, w], bf16, tag="y")
                        if dj % 2 == 0:
                            ysrc = ibA[di][:, :, dj + 2 : dj + 2 + w]
                        else:
                            ysrc = ibB[di][:, :, dj + 1 : dj + 1 + w]
                        nc.vector.tensor_tensor(
                            yp[:], wp[:, :, -cP : -cP + w], ysrc, Alu.mult
                        )

                        # ---- 4 matmuls per channel ----
                        s_dir = 2
                        s_sh = 2 - di
                        first = mm_i == 0
                        last = mm_i == n_mm - 2  # this pair adds 2 mms per target
                        w_merged = dj == 0  # dir+shifted share the rhs window
    # (merged lhsT idx: di=1 -> 3, di=2 -> 4)
                        for ch in range(c):
                            nc.tensor.matmul(
                                pw[:, ch, 0:w],
                                shifts[:, (2 + di) if w_merged else s_dir, :],
                                wp[:, ch, -cP : -cP + w],
                                start=first,
                                stop=last if w_merged else False,
                            )
                            nc.tensor.matmul(
                                pa[:, ch, 0:w],
                                shifts[:, s_dir, :],
                                yp[:, ch, :],
                                start=first,
                                stop=False,
                            )
                        for ch in range(c):
                            if not w_merged:
                                nc.tensor.matmul(
                                    pw[:, ch, 0:w],
                                    shifts[:, s_sh, :],
                                    wp[:, ch, -dj - cP : -dj - cP + w],
                                    start=False,
                                    stop=last,
                                )
                            nc.tensor.matmul(
                                pa[:, ch, 0:w],
                                shifts[:, s_sh, :],
                                zp[:, ch, -dj - cP : -dj - cP + w],
                                start=False,
                                stop=last,
                            )
                        mm_i += 2

                # ---- finalize: out = (pa + center) * exp(-ln(pw + 1)) ----
                lnv = finals.tile([NOUT, c, w], f32, tag="lnv")
                nc.scalar.activation(lnv[:], pw[:, :, 0:w], AF.Ln, bias=1.0)
                rec = finals.tile([NOUT, c, w], f32, tag="rec")
                nc.scalar.activation(rec[:], lnv[:], AF.Exp, scale=-1.0)
                acct = finals.tile([NOUT, c, w], f32, tag="acct")
                nc.vector.tensor_tensor(
                    acct[:], pa[:, :, 0:w], ifs[2][0:NOUT, :, 2 : 2 + w], Alu.add
                )
                res = finals.tile([NOUT, c, w], f32, tag="res")
                nc.vector.tensor_tensor(res[:], acct[:], rec[:], Alu.mult)
                # overlap band: only write rows not already written by the
                # previous band (avoids DRAM WAW serialization)
                oo = 0 if r0 == bands[0] else max(0, prev_end - r0)
                nc.sync.dma_start(
                    out_d[bi, :, r0 + oo : r0 + NOUT, :].rearrange("c r n -> r c n"),
                    res[oo:NOUT],
                )
                prev_end = r0 + NOUT
    return nc


def _shift_mats():
    s = np.zeros((NG, 5, NOUT), dtype=ml_dtypes.bfloat16)
    for k in range(3):
        for m in range(NOUT):
            s[m + k, k, m] = 1.0
    # merged direct+shifted for dj=0 planes: idx 3 = L2+L1, idx 4 = L2+L0
    s[:, 3] = s[:, 2] + s[:, 1]
    s[:, 4] = s[:, 2] + s[:, 0]
    return s


def make_program(spatial_kernel):
    biases = {}
    for (di, dj) in PAIRS:
        v = float(np.float32(np.log(np.float32(spatial_kernel[2 + di, 2 + dj]))))
        biases[(di, dj)] = v
    nc = bacc.Bacc("TRN2", target_bir_lowering=False, debug=False)
    build_bilateral(nc, biases)
    nc.compile()
    return nc


def kernel(images, spatial_kernel):
    images = np.asarray(images, dtype=np.float32)
    spatial_kernel = np.asarray(spatial_kernel, dtype=np.float32)
    nc = make_program(spatial_kernel)
    shifts = _shift_mats()
    in_maps = [
        {"images": images[i * B_SH : (i + 1) * B_SH], "shifts": shifts}
        for i in range(N_CORES)
    ]
    res = run_bass_kernel_spmd(nc, in_maps, core_ids=list(range(N_CORES)))
    return np.concatenate([res.results[i]["out"] for i in range(N_CORES)], axis=0)



# revision 4
# speedup vs baseline: 1.1273x; 1.0331x over previous
"""Bilateral anti-alias filter on Trainium2, 8-core data parallel.

Full inputs: images [16,3,512,512] f32, spatial_kernel [5,5] f32.
Shards the batch over 8 NeuronCores (2 images each), runs a Bass/Tile
kernel per core, gathers the full output.

Math (per pixel, K=5, sigma_i=0.1):
  w_t = s_t * exp(-50*(p_t - c)^2),  out = sum(w_t*p_t)/(sum(w_t)+eps)
Pair symmetry: w for tap -(di,dj) at pixel r equals w for tap +(di,dj)
at pixel r-(di,dj), so only 12 weight planes are computed.

U-trick: with d_v = p(.+v) - p and U_v = w_v*d_v,
  num = p*den + sum_v [U_v(r) - U_v(r-v)]   =>   out = p + pu/(1+pw)
so per pair only ONE product (U) is needed on the Vector engine; the
+/-1 signs and row shifts are folded into 0/+-1 matrices fed to the
TensorEngine, which accumulates pw (denominator) and pu (numerator
correction) into PSUM.  Column shifts are SBUF slices (free).

Engine budget per 128-row band tile (~30us): DVE ~30us (subs, U, 2
group squares, casts, finalize), ACT ~30us (4 group squares, exps,
Ln/Exp reciprocal; one table set, preloaded), PE ~132 matmuls.
GpSimd is kept idle: it shares an exclusive SBUF port pair with the
DVE's second read port, so any GpSimd op blocks every tensor_tensor.
"""
import sys

sys.path.insert(0, "/opt/trn_rl_repo")

import numpy as np
import ml_dtypes
from contextlib import ExitStack

import concourse.bass as bass
import concourse.tile as tile
from concourse import bacc, mybir
from concourse.bass_utils import run_bass_kernel_spmd

f32 = mybir.dt.float32
bf16 = mybir.dt.bfloat16
AF = mybir.ActivationFunctionType
Alu = mybir.AluOpType

N_CORES = 8
B_FULL, C, H, W = 16, 3, 512, 512
B_SH = B_FULL // N_CORES  # 2 images per core
KK = 5
PAD = KK // 2  # 2
INV2SIG2 = 1.0 / (2.0 * 0.1 * 0.1)  # 50.0

# 12 pairs (di, dj) with di >= 0, lexicographically positive
PAIRS = [
    (0, 1), (0, 2),
    (1, -2), (1, -1), (1, 0), (1, 1), (1, 2),
    (2, -2), (2, -1), (2, 0), (2, 1), (2, 2),
]

# groups pair taps with EQUAL spatial weight (same di^2+dj^2) so one
# exp bias serves the whole group
GROUPS = [
    [(0, 1), (1, 0)], [(1, -1), (1, 1)], [(0, 2), (2, 0)],
    [(1, -2), (1, 2)], [(2, -1), (2, 1)], [(2, -2), (2, 2)],
]
SQ_DVE_GROUPS = {1, 4}  # these groups square on DVE; rest on ACT
NOUT = 124  # output rows per tile
NG = 128    # grid partitions (= NOUT + 4)
WB = W + 4  # 516: padded col buffer, idx j <-> col j-2

# shift-matrix variant indices (lhsT slices of the shifts tensor)
SH_S0, SH_S1, SH_S2 = 0, 1, 2
SH_S2pS1, SH_S2pS0 = 3, 4          # pw merged dj=0 (di=1, di=2)
SH_nS0, SH_nS1, SH_nS2 = 5, 6, 7   # negated, for pu shifted terms
SH_S2mS1, SH_S2mS0 = 8, 9          # pu merged dj=0
N_SHIFT = 10


def _row_bands(h):
    """Tile start rows: uniform NOUT-row bands; last band overlaps upward."""
    bands = list(range(0, h - NOUT + 1, NOUT))
    if bands[-1] != h - NOUT:
        bands.append(h - NOUT)
    return bands


def _reflect_runs(v0, v1, h):
    """Split virtual row range [v0, v1] into runs of physical rows.
    Returns list of (p_offset, phys_start, count, step) with step +-1."""
    runs = []
    v = v0
    while v <= v1:
        if v < 0:
            e = min(-1, v1)
            runs.append((v - v0, -v, e - v + 1, -1))
            v = e + 1
        elif v >= h:
            e = v1
            runs.append((v - v0, 2 * h - 2 - v, e - v + 1, -1))
            v = e + 1
        else:
            e = min(h - 1, v1)
            runs.append((v - v0, v, e - v + 1, 1))
            v = e + 1
    return runs


def build_bilateral(nc, biases, h=H, w=W, b_sh=B_SH, c=C):
    """Emit the per-core program into nc (a Bacc). biases[(di,dj)] = ln s."""
    wb = w + 4
    img_d = nc.dram_tensor("images", [b_sh, c, h, w], f32, kind="ExternalInput").ap()
    shifts_d = nc.dram_tensor(
        "shifts", [NG, N_SHIFT, NOUT], bf16, kind="ExternalInput"
    ).ap()
    out_d = nc.dram_tensor("out", [b_sh, c, h, w], f32, kind="ExternalOutput").ap()

    # const APs for activation biases
    for val in sorted(set(biases.values()) | {1.0}):
        key = (f32, val)
        if key not in nc.const_aps.aps:
            t = nc.alloc_sbuf_tensor(f"cbias-{val}", [128, 1], f32)
            nc.gpsimd.memset(t.ap(), val)
            nc.const_aps.aps[key] = t.ap()
    nc.all_engine_barrier()

    # Pre-place one ACT table load for the set containing Square+Exp+Ln;
    # otherwise the compiler's greedy chooser thrashes between
    # exp_and_others and natural_log (2 reloads/tile, ~2.7us each).
    from concourse.hw_specs import get_activation_tables

    set_names = list(get_activation_tables(nc.m.arch))
    nc.scalar.add_instruction(
        mybir.InstLoadActFuncSet(
            name=nc.get_next_instruction_name(),
            act_func_set_id=set_names.index("natural_log_exp_and_others"),
            ins=[],
            outs=[],
        )
    )

    bands = _row_bands(h)

    with tile.TileContext(nc) as tc, ExitStack() as ctx:
        consts = ctx.enter_context(tc.tile_pool(name="consts", bufs=1))
        imgs_f = ctx.enter_context(tc.tile_pool(name="imgs_f", bufs=2))
        imgs_b = ctx.enter_context(tc.tile_pool(name="imgs_b", bufs=2))
        planes = ctx.enter_context(tc.tile_pool(name="planes", bufs=2))
        prods = ctx.enter_context(tc.tile_pool(name="prods", bufs=2))
        finals = ctx.enter_context(tc.tile_pool(name="finals", bufs=1))
        psums = ctx.enter_context(tc.tile_pool(name="psums", bufs=1, space="PSUM"))

        shifts = consts.tile([NG, N_SHIFT, NOUT], bf16)
        nc.sync.dma_start(shifts[:], shifts_d[:])

        for bi in range(b_sh):
            for r0 in bands:
                # ---- load 3 row-shifted reflect-padded image copies ----
                ifs = []
                for s in range(3):
                    t = imgs_f.tile([NG, c, wb], f32, tag=f"i{s}f")
                    refl_rows = []
                    for (po, ps, cnt, step) in _reflect_runs(
                        r0 - 2 + s, r0 - 2 + s + NG - 1, h
                    ):
                        if step == 1:
                            src = img_d[bi, :, ps : ps + cnt, :]
                            nc.sync.dma_start(
                                t[po : po + cnt, :, 2 : 2 + w],
                                src.rearrange("c r n -> r c n"),
                            )
                        else:
                            for k in range(cnt):
                                refl_rows.append((po + k, ps - k))
                    # reflect rows duplicate rows already in the tile
                    for (p_dst, phys) in refl_rows:
                        p_src = phys - (r0 - 2 + s)
                        nc.sync.dma_start(
                            t[p_dst : p_dst + 1, :, 2 : 2 + w],
                            t[p_src : p_src + 1, :, 2 : 2 + w],
                        )
                    # reflect pad cols: tiny DVE copies (NOT GpSimd: GpSimd
                    # ops block the shared SBUF port the DVE TTs need)
                    for (j, jsrc) in ((0, 4), (1, 3), (2 + w, w), (3 + w, w - 1)):
                        nc.vector.tensor_copy(
                            t[:, :, j : j + 1], t[:, :, jsrc : jsrc + 1]
                        )
                    ifs.append(t)

                # bf16 copies: A (cast), B (A shifted 1 col, via DMA)
                ibA, ibB = [], []
                for s in range(3):
                    a = imgs_b.tile([NG, c, wb], bf16, tag=f"i{s}bA")
                    nc.vector.tensor_copy(a[:], ifs[s][:])
                    ibA.append(a)
                    b = imgs_b.tile([NG, c, wb], bf16, tag=f"i{s}bB")
                    nc.sync.dma_start(b[:, :, 0 : wb - 1], a[:, :, 1:wb])
                    ibB.append(b)

                # ---- PSUM accumulators ----
                pw = psums.tile([NOUT, c, 512], f32, tag="pw")
                pu = psums.tile([NOUT, c, 512], f32, tag="pu")

                n_con = 2 * len(PAIRS) - 2  # pw/pu contributions per channel
                con_i = 0
                for g_i, grp in enumerate(GROUPS):
                    G = len(grp)
                    dg = planes.tile([NG, G * c, w + 2], bf16, tag="d")
                    for gi, (di, dj) in enumerate(grp):
                        cP = -2 if dj > 0 else 0
                        if dj % 2 == 0:
                            dsrc = ibA[di][:, :, cP + dj + 2 : cP + dj + 4 + w]
                        else:
                            dsrc = ibB[di][:, :, cP + dj + 1 : cP + dj + 3 + w]
                        nc.vector.tensor_tensor(
                            dg[:, gi * c : (gi + 1) * c, :], dsrc,
                            ibA[0][:, :, cP + 2 : cP + 4 + w], Alu.subtract,
                        )
                    sqg = planes.tile([NG, G * c, w + 2], bf16, tag="sq")
                    if g_i in SQ_DVE_GROUPS:
                        nc.vector.tensor_tensor(sqg[:], dg[:], dg[:], Alu.mult)
                    else:
                        nc.scalar.activation(sqg[:], dg[:], AF.Square)
                    wg = planes.tile([NG, G * c, w + 2], bf16, tag="w")
                    nc.scalar.activation(
                        wg[:], sqg[:], AF.Exp,
                        bias=biases[grp[0]], scale=-INV2SIG2,
                    )
                    ug = prods.tile([NG, G * c, w + 2], bf16, tag="u")
                    nc.vector.tensor_tensor(ug[:], wg[:], dg[:], Alu.mult)

                    for gi, (di, dj) in enumerate(grp):
                        cP = -2 if dj > 0 else 0
                        wp = wg[:, gi * c : (gi + 1) * c, :]
                        up = ug[:, gi * c : (gi + 1) * c, :]
                        first = con_i == 0
                        if dj == 0:
                            # merged direct+shifted (same rhs window, cP=0)
                            last = con_i == n_con - 1
                            for ch in range(c):
                                nc.tensor.matmul(
                                    pw[:, ch, 0:w],
                                    shifts[:, SH_S2pS1 if di == 1 else SH_S2pS0, :],
                                    wp[:, ch, 0:w],
                                    start=first, stop=last,
                                )
                                nc.tensor.matmul(
                                    pu[:, ch, 0:w],
                                    shifts[:, SH_S2mS1 if di == 1 else SH_S2mS0, :],
                                    up[:, ch, 0:w],
                                    start=first, stop=last,
                                )
                            con_i += 1
                        else:
                            last = con_i == n_con - 2
                            sh_neg = (SH_nS2, SH_nS1, SH_nS0)[di]
                            sh_pos = (SH_S2, SH_S1, SH_S0)[di]
                            for ch in range(c):
                                # direct: +w, +U at row offset 2, col 0
                                nc.tensor.matmul(
                                    pw[:, ch, 0:w],
                                    shifts[:, SH_S2, :],
                                    wp[:, ch, -cP : -cP + w],
                                    start=first, stop=False,
                                )
                                nc.tensor.matmul(
                                    pu[:, ch, 0:w],
                                    shifts[:, SH_S2, :],
                                    up[:, ch, -cP : -cP + w],
                                    start=first, stop=False,
                                )
                            for ch in range(c):
                                # shifted: +w, -U at row offset 2-di, col -dj
                                nc.tensor.matmul(
                                    pw[:, ch, 0:w],
                                    shifts[:, sh_pos, :],
                                    wp[:, ch, -dj - cP : -dj - cP + w],
                                    start=False, stop=last,
                                )
                                nc.tensor.matmul(
                                    pu[:, ch, 0:w],
                                    shifts[:, sh_neg, :],
                                    up[:, ch, -dj - cP : -dj - cP + w],
                                    start=False, stop=last,
                                )
                            con_i += 2

                # ---- finalize: out = p + pu * exp(-ln(pw + 1)) ----
                lnv = finals.tile([NOUT, c, w], f32, tag="lnv")
                nc.scalar.activation(lnv[:], pw[:, :, 0:w], AF.Ln, bias=1.0)
                rec = finals.tile([NOUT, c, w], f32, tag="rec")
                nc.scalar.activation(rec[:], lnv[:], AF.Exp, scale=-1.0)
                acct = finals.tile([NOUT, c, w], f32, tag="acct")
                nc.vector.tensor_tensor(
                    acct[:], pu[:, :, 0:w], rec[:], Alu.mult
                )
                res = finals.tile([NOUT, c, w], f32, tag="res")
                nc.vector.tensor_tensor(
                    res[:], acct[:], ifs[2][0:NOUT, :, 2 : 2 + w], Alu.add
                )
                # overlap band: only write rows not already written
                oo = 0 if r0 == bands[0] else max(0, prev_end - r0)
                nc.sync.dma_start(
                    out_d[bi, :, r0 + oo : r0 + NOUT, :].rearrange("c r n -> r c n"),
                    res[oo:NOUT],
                )
                prev_end = r0 + NOUT
    return nc


def _shift_mats():
    s = np.zeros((NG, N_SHIFT, NOUT), dtype=ml_dtypes.bfloat16)
    for k in range(3):
        for m in range(NOUT):
            s[m + k, k, m] = 1.0
    s[:, SH_S2pS1] = s[:, 2] + s[:, 1]
    s[:, SH_S2pS0] = s[:, 2] + s[:, 0]
    s[:, SH_nS0] = -s[:, 0]
    s[:, SH_nS1] = -s[:, 1]
    s[:, SH_nS2] = -s[:, 2]
    s[:, SH_S2mS1] = s[:, 2] - s[:, 1]
    s[:, SH_S2mS0] = s[:, 2] - s[:, 0]
    return s


def make_program(spatial_kernel):
    biases = {}
    for (di, dj) in PAIRS:
        v = float(np.float32(np.log(np.float32(spatial_kernel[2 + di, 2 + dj]))))
        biases[(di, dj)] = v
    nc = bacc.Bacc("TRN2", target_bir_lowering=False, debug=False)
    build_bilateral(nc, biases)
    nc.compile()
    return nc


def kernel(images, spatial_kernel):
    images = np.asarray(images, dtype=np.float32)
    spatial_kernel = np.asarray(spatial_kernel, dtype=np.float32)
    nc = make_program(spatial_kernel)
    shifts = _shift_mats()
    in_maps = [
        {"images": images[i * B_SH : (i + 1) * B_SH], "shifts": shifts}
        for i in range(N_CORES)
    ]
    res = run_bass_kernel_spmd(nc, in_maps, core_ids=list(range(N_CORES)))
    return np.concatenate([res.results[i]["out"] for i in range(N_CORES)], axis=0)
